# revision 3
# baseline (speedup 1.0000x reference)
"""Trainium2 Bass kernel for nn_CGLayer (gnn_message_passing).

Contract: kernel(**inputs) takes FULL inputs (as reference.setup_inputs()),
returns FULL output [8,128,1,16,9] f32. Internally: data-parallel over the
batch dim across 8 NeuronCores; per core one batch element.

Algebraic reduction (exact):
  X   = conn @ vertices                  (message passing, per batch)
  Y   = mix_nl(cg(X, X))                 (per-node quadratic in X)
  S   = sum_j sph[:, j, :]               (neighbor sum commutes through the
  Z   = mix_rel(cg(Y, S))                 relative-CG stage: x-side is
  out = Z / sqrt(sum Z^2 / 16)            j-independent)

Device pipeline per core (features-on-free "i-partition" layout for
products; PE transposes flip to slot-partition layout for the big mix):
  A:  X[i,144]    = matmul(lhsT=connT, rhs=vcat)
  S:  Ssum[i,9]   = reduce_j(sph);  S[9,i] = PE-transpose
  B:  P[i,17920]  = 14 stride-0 DVE tensor_tensor ops (pair products,
                    s-group padded slot layout; W2 host-combined CG x w_nl)
      PT chunks   = 140 PE transposes + PSUM->SBUF copies
      Y[144,i]    = 140 matmuls lhsT=W2-chunks (PSUM-accumulated per s-group)
  C:  P2          = Y (piece layout) * replicated S rows (DVE)
      Z[144,i]    = 36 matmuls lhsT=W3-chunks
Host epilogue: gather, unpack e=(l,c',k), global normalization per l.
"""
import numpy as np
from math import factorial, sqrt

MAXL = 2
CH = 16
NN = 128
NB = 8
LDIM = [1, 3, 5]
FOFF = [0, 16, 64]
NF = 144
TAU_NL = [768, 1536, 1536]
TAU_REL = [48, 96, 96]

# ------------------------------------------------------------- CG tables
def _cg_coeff(j1, m1, j2, m2, j3, m3):
    if m3 != m1 + m2:
        return 0.0
    pre = sqrt((2 * j3 + 1) * factorial(j3 + j1 - j2) * factorial(j3 - j1 + j2)
               * factorial(j1 + j2 - j3) / factorial(j1 + j2 + j3 + 1))
    pre *= sqrt(factorial(j3 + m3) * factorial(j3 - m3) * factorial(j1 - m1)
                * factorial(j1 + m1) * factorial(j2 - m2) * factorial(j2 + m2))
    s = 0.0
    vmin = max(0, j2 - j3 - m1, j1 - j3 + m2)
    vmax = min(j1 + j2 - j3, j1 - m1, j2 + m2)
    for v in range(vmin, vmax + 1):
        s += (-1) ** v / (factorial(v) * factorial(j1 + j2 - j3 - v)
                          * factorial(j1 - m1 - v) * factorial(j2 + m2 - v)
                          * factorial(j3 - j2 + m1 + v) * factorial(j3 - j1 - m2 + v))
    return pre * s


def _cg_matrix(l1, l2, l):
    M = np.zeros((2 * l1 + 1, 2 * l2 + 1, 2 * l + 1))
    for m1 in range(-l1, l1 + 1):
        for m2 in range(-l2, l2 + 1):
            if -l <= m1 + m2 <= l:
                M[m1 + l1, m2 + l2, m1 + m2 + l] = _cg_coeff(l1, m1, l2, m2, l, m1 + m2)
    return M


def _valid_pairs(l):
    return [(l1, l2) for l1 in range(3) for l2 in range(3)
            if abs(l1 - l2) <= l <= l1 + l2]

# ----------------------------------------------------- stage-B slot layout
Q_COMBOS = [(l1, l2, m1) for l1 in range(3) for l2 in range(l1, 3)
            for m1 in range(2 * l1 + 1)]
NQ = len(Q_COMBOS)                       # 14
GRP = NQ * 256                           # 3584
NSLOT = 5 * GRP                          # 17920
NCHUNK = NSLOT // 128                    # 140
GCH = GRP // 128                         # 28 chunks per s-group


def _sgroup_cols(g):
    st = g - 2
    return [(l, cp) for l in range(3) if abs(st) <= l for cp in range(CH)]

SG_NCOL = [len(_sgroup_cols(g)) for g in range(5)]      # [16,32,48,32,16]
YOFF = np.concatenate([[0], np.cumsum(SG_NCOL)])        # piece row offsets


def _sg_lblock_col(g, l):
    st = g - 2
    return 16 * sum(1 for lp in range(l) if abs(st) <= lp)


def _product_ops():
    ops = []
    for qi, (l1, l2, m1) in enumerate(Q_COMBOS):
        mt1 = m1 - l1
        m2_lo = max(0, -2 - mt1 + l2)
        m2_hi = min(2 * l2, 2 - mt1 + l2)
        n_m2 = m2_hi - m2_lo + 1
        g0 = mt1 + (m2_lo - l2) + 2
        ops.append(dict(l1=l1, l2=l2, m1=m1, m2_lo=m2_lo, n_m2=n_m2,
                        out_off=g0 * GRP + qi * 256))
    return ops

PRODUCT_OPS = _product_ops()
_QIDX = {q: i for i, q in enumerate(Q_COMBOS)}
_CAR, _DAR = np.meshgrid(np.arange(16), np.arange(16), indexing="ij")


def _assemble_W2(w_nl):
    """W2[NSLOT, 48] f64->f32: combined CG x w_nl, s-grouped slot layout."""
    W2 = np.zeros((NSLOT, 48))
    for l in range(3):
        off = 0
        for (p1, p2) in _valid_pairs(l):
            Cg = _cg_matrix(p1, p2, l)
            wl = np.asarray(w_nl[l], np.float64)
            for m1 in range(2 * p1 + 1):
                for m2 in range(2 * p2 + 1):
                    st = (m1 - p1) + (m2 - p2)
                    if abs(st) > l:
                        continue
                    gc = Cg[m1, m2, st + l]
                    if gc == 0.0:
                        continue
                    g = st + 2
                    if p1 <= p2:
                        slots = g * GRP + _QIDX[(p1, p2, m1)] * 256 + _CAR * 16 + _DAR
                    else:
                        slots = g * GRP + _QIDX[(p2, p1, m2)] * 256 + _DAR * 16 + _CAR
                    t = off + _CAR * 16 + _DAR
                    c0 = _sg_lblock_col(g, l)
                    W2[slots.ravel(), c0:c0 + 16] += gc * wl[t.ravel(), :]
            off += 256
    return W2.astype(np.float32)


def _assemble_W3(w_rel):
    """W3[9*144, 144]: contraction P2[(n,a), i] -> Z[e, i]; a = Y piece row."""
    SOFF = [0, 1, 4]
    W3 = np.zeros((9 * 144, 144))
    ar = np.arange(16)
    for l in range(3):
        off = 0
        for (p1, p2) in _valid_pairs(l):
            Cg = _cg_matrix(p1, p2, l)
            wr = np.asarray(w_rel[l], np.float64)
            for m1 in range(2 * p1 + 1):
                for m2 in range(2 * p2 + 1):
                    st = (m1 - p1) + (m2 - p2)
                    if abs(st) > l:
                        continue
                    gc = Cg[m1, m2, st + l]
                    if gc == 0.0:
                        continue
                    gY = (m1 - p1) + 2
                    a0 = YOFF[gY] + _sg_lblock_col(gY, p1)
                    rows = (SOFF[p2] + m2) * 144 + a0 + ar
                    cols = FOFF[l] + (st + l) + ar * LDIM[l]
                    W3[np.ix_(rows, cols)] += gc * wr[off:off + 16, :]
            off += 16
    return W3.astype(np.float32)

# ------------------------------------------------------------ bass builder
_NC_CACHE = {}


def _build_nc(debug=False):
    import concourse.bacc as bacc
    import concourse.bass as bass
    import concourse.tile as tile
    from concourse import mybir
    from concourse.masks import make_identity

    f32 = mybir.dt.float32
    nc = bacc.Bacc()
    d_connT = nc.declare_dram_parameter("connT", [128, 128], f32, isOutput=False)
    d_vcat = nc.declare_dram_parameter("vcat", [128, NF], f32, isOutput=False)
    d_sph = nc.declare_dram_parameter("sph", [128, 128 * 9], f32, isOutput=False)
    d_w2 = nc.declare_dram_parameter("w2", [128, NCHUNK * 48], f32, isOutput=False)
    d_w3g = nc.declare_dram_parameter("w3g", [48, 9 * 5 * 144], f32, isOutput=False)
    d_zout = nc.declare_dram_parameter("zout", [144, 128], f32, isOutput=True)
    if debug:
        d_dbgx = nc.declare_dram_parameter("dbgx", [128, NF], f32, isOutput=True)
        d_dbgs = nc.declare_dram_parameter("dbgs", [9, 128], f32, isOutput=True)
        d_dbgp = nc.declare_dram_parameter("dbgp", [128, NSLOT], f32, isOutput=True)
        d_dbgy = nc.declare_dram_parameter("dbgy", [48, 5 * 128], f32, isOutput=True)
        d_dbgr = nc.declare_dram_parameter("dbgr", [48, 128], f32, isOutput=True)
        d_dbgp2 = nc.declare_dram_parameter("dbgp2", [48, 5 * 128], f32, isOutput=True)

    def vap(t, doff, freedims):
        base = t[:] if not isinstance(t, bass.AP) else t
        return bass.AP(tensor=base.tensor, offset=base.offset + doff,
                       ap=[list(base.ap[0])] + [list(d) for d in freedims])

    with tile.TileContext(nc) as tc:
      with (
        tc.tile_pool(name="big", bufs=1) as big,
        tc.tile_pool(name="sb", bufs=1) as sb,
        tc.tile_pool(name="pt", bufs=4) as ptp,
        tc.tile_pool(name="p2", bufs=4) as p2p,
        tc.tile_pool(name="ps_x", bufs=1, space="PSUM") as ps_x,
        tc.tile_pool(name="ps_t", bufs=2, space="PSUM") as ps_t,
        tc.tile_pool(name="ps_y", bufs=1, space="PSUM") as ps_y,
        tc.tile_pool(name="ps_r", bufs=1, space="PSUM") as ps_r,
        tc.tile_pool(name="ps_z", bufs=1, space="PSUM") as ps_z,
      ):
        # ---- input DMAs
        connT = sb.tile([128, 128], f32)
        nc.sync.dma_start(out=connT, in_=d_connT[:, :])
        vcat = sb.tile([128, NF], f32)
        nc.sync.dma_start(out=vcat, in_=d_vcat[:, :])
        sph = big.tile([128, 128 * 9], f32)
        for q in range(4):
            nc.sync.dma_start(out=sph[:, q * 288:(q + 1) * 288],
                              in_=d_sph[:, q * 288:(q + 1) * 288])
        w2 = big.tile([128, NCHUNK, 48], f32)
        for q in range(4):
            s = q * (NCHUNK // 4) * 48
            e = (q + 1) * (NCHUNK // 4) * 48
            nc.sync.dma_start(out=vap(w2, s, [[1, e - s]]),
                              in_=d_w2[:, s:e])
        w3g = sb.tile([48, 9, 5, 144], f32)
        nc.sync.dma_start(
            out=w3g, in_=d_w3g[:, :].rearrange("p (n g e) -> p n g e", n=9, g=5))
        ident = sb.tile([128, 128], f32)
        make_identity(nc, ident)

        # ---- stage A: X[i, feat] = connT.T @ vcat
        x_ps = ps_x.tile([128, NF], f32, tag="misc", name="x_ps")
        nc.tensor.matmul(x_ps, connT, vcat, start=True, stop=True)
        X = sb.tile([128, NF], f32)
        nc.scalar.activation(X, x_ps, mybir.ActivationFunctionType.Copy)

        # ---- stage S: Ssum[i, 9] = sum_j sph; S[9, i]
        ssum = sb.tile([128, 9], f32)
        nc.vector.tensor_reduce(
            ssum, vap(sph, 0, [[1, 9], [9, 128]]),
            mybir.AxisListType.X, mybir.AluOpType.add)
        s_ps = ps_x.tile([128, NF], f32, tag="misc", name="s_ps")[0:9, 0:128]
        nc.tensor.transpose(s_ps, ssum, ident)
        S = sb.tile([9, 128], f32)
        nc.scalar.activation(S, s_ps, mybir.ActivationFunctionType.Copy)
        sel9 = sb.tile([9, 9, 128], f32)
        nc.gpsimd.memset(sel9, 0.0)
        nc.gpsimd.affine_select(
            out=sel9, in_=sel9, compare_op=mybir.AluOpType.not_equal,
            fill=1.0, base=0, pattern=[[-1, 9], [0, 128]], channel_multiplier=1)

        # ---- stage B products: P[i, NSLOT]
        P = big.tile([128, NSLOT], f32)
        for h in range(2):  # memset split across two engines
            eng = nc.gpsimd if h == 0 else nc.vector
            eng.memset(P[:, h * (NSLOT // 2):(h + 1) * (NSLOT // 2)], 0.0)
        for op in PRODUCT_OPS:
            l1, l2, m1 = op["l1"], op["l2"], op["m1"]
            nm2 = op["n_m2"]
            nc.vector.tensor_tensor(
                out=vap(P, op["out_off"], [[GRP, nm2], [16, 16], [1, 16]]),
                in0=vap(X, FOFF[l1] + m1, [[0, nm2], [LDIM[l1], 16], [0, 16]]),
                in1=vap(X, FOFF[l2] + op["m2_lo"],
                        [[1, nm2], [0, 16], [LDIM[l2], 16]]),
                op=mybir.AluOpType.mult)

        # ---- stage B transposes + mix: Y pieces, PSUM-accumulated
        ymix = ps_y.tile([48, 5, 128], f32)
        cp_engines = [nc.scalar, nc.vector]
        for g in range(5):
            ncol = SG_NCOL[g]
            for ch in range(GCH):
                t_ps = ps_t.tile([128, 128], f32)
                nc.tensor.transpose(
                    t_ps, P[:, (g * GCH + ch) * 128:(g * GCH + ch + 1) * 128], ident)
                pt = ptp.tile([128, 128], f32)
                eng = cp_engines[(g * GCH + ch) % 2]
                if eng is nc.scalar:
                    nc.scalar.activation(pt, t_ps, mybir.ActivationFunctionType.Copy)
                else:
                    eng.tensor_copy(out=pt, in_=t_ps)
                nc.tensor.matmul(ymix[0:ncol, g, :], w2[:, g * GCH + ch, 0:ncol], pt,
                                 start=(ch == 0), stop=(ch == GCH - 1))

        # Y pieces -> SBUF, packed [48, 5, 128]
        ysb = sb.tile([48, 5, 128], f32)
        nc.scalar.activation(ysb, ymix, mybir.ActivationFunctionType.Copy)

        if debug:
            nc.sync.dma_start(out=d_dbgx[:, :], in_=X)
            nc.sync.dma_start(out=d_dbgs[:, :], in_=S)
            for q in range(4):
                nc.sync.dma_start(out=d_dbgp[:, q * 4480:(q + 1) * 4480],
                                  in_=P[:, q * 4480:(q + 1) * 4480])
            nc.sync.dma_start(out=d_dbgy[:, :], in_=ysb)

        # ---- stage C: P2 = Y * rep(S_n); Z = sum_n W3_n.T @ P2_n
        z_hi = ps_z.tile([128, 128], f32)
        z_lo = ps_z.tile([16, 128], f32)
        for n in range(9):
            rep = ps_r.tile([48, 128], f32)
            nc.tensor.matmul(rep, sel9[:, n, 0:48], S, start=True, stop=True)
            p2 = p2p.tile([48, 5, 128], f32)
            nc.vector.tensor_tensor(
                out=p2,
                in0=vap(rep, 0, [[0, 5], [1, 128]]),
                in1=ysb, op=mybir.AluOpType.mult)
            if debug and n == 0:
                rep_sb = sb.tile([48, 128], f32)
                nc.vector.tensor_copy(out=rep_sb, in_=rep)
                nc.sync.dma_start(out=d_dbgr[:, :], in_=rep_sb)
                nc.sync.dma_start(out=d_dbgp2[:, :], in_=p2)
            for g in range(5):
                ncol = SG_NCOL[g]
                nc.tensor.matmul(z_hi, w3g[0:ncol, n, g, 0:128], p2[0:ncol, g, :],
                                 start=(n == 0 and g == 0), stop=(n == 8 and g == 4))
                nc.tensor.matmul(z_lo, w3g[0:ncol, n, g, 128:144], p2[0:ncol, g, :],
                                 start=(n == 0 and g == 0), stop=(n == 8 and g == 4))

        zs_hi = sb.tile([128, 128], f32)
        zs_lo = sb.tile([16, 128], f32)
        nc.scalar.activation(zs_hi, z_hi, mybir.ActivationFunctionType.Copy)
        nc.scalar.activation(zs_lo, z_lo, mybir.ActivationFunctionType.Copy)
        nc.sync.dma_start(out=d_zout[0:128, :], in_=zs_hi)
        nc.sync.dma_start(out=d_zout[128:144, :], in_=zs_lo)

    nc.compile()
    return nc

# ------------------------------------------------------------- host entry
LAST_RESULT = {}


def _get_nc():
    if "nc" not in _NC_CACHE:
        _NC_CACHE["nc"] = _build_nc()
    return _NC_CACHE["nc"]


def kernel(vertices_0, vertices_1, vertices_2, connectivity,
           sph_0, sph_1, sph_2,
           w_nl_0, w_nl_1, w_nl_2,
           w_rel_0, w_rel_1, w_rel_2):
    from concourse.bass_utils import run_bass_kernel_spmd

    f = np.float32
    verts = [np.asarray(v, f) for v in (vertices_0, vertices_1, vertices_2)]
    sphs = [np.asarray(s, f) for s in (sph_0, sph_1, sph_2)]
    conn = np.asarray(connectivity)
    W2 = _assemble_W2([np.asarray(w, f) for w in (w_nl_0, w_nl_1, w_nl_2)])
    W3 = _assemble_W3([np.asarray(w, f) for w in (w_rel_0, w_rel_1, w_rel_2)])
    # pack to SBUF-ready layouts (shared across cores)
    w2p = np.ascontiguousarray(
        W2.reshape(NCHUNK, 128, 48).transpose(1, 0, 2).reshape(128, NCHUNK * 48))
    W3r = W3.reshape(9, 144, 144)
    w3g = np.zeros((48, 9, 5, 144), np.float32)
    for g in range(5):
        w3g[0:SG_NCOL[g], :, g, :] = W3r[:, YOFF[g]:YOFF[g] + SG_NCOL[g], :].transpose(1, 0, 2)
    w3g = np.ascontiguousarray(w3g.reshape(48, 9 * 5 * 144))

    in_maps = []
    for b in range(NB):
        connT = np.ascontiguousarray(conn[b].astype(f).T)
        vcat = np.concatenate([v[b].reshape(128, -1) for v in verts], axis=1)
        sph_cat = np.concatenate([s[b][:, :, 0, :] for s in sphs], axis=-1)
        in_maps.append(dict(connT=connT, vcat=np.ascontiguousarray(vcat),
                            sph=np.ascontiguousarray(sph_cat.reshape(128, 128 * 9)),
                            w2=w2p, w3g=w3g))

    res = run_bass_kernel_spmd(_get_nc(), in_maps, list(range(NB)))
    LAST_RESULT["res"] = res
    Z = np.stack([res.results[b]["zout"] for b in range(NB)])   # [8, 144, 128]

    # host epilogue: unpack e=(l,cp,k) rows, global per-l normalization
    out = np.zeros((NB, 128, 1, 16, 9), dtype=f)
    koff = [0, 1, 4]
    for l in range(3):
        blk = Z[:, FOFF[l]:FOFF[l] + 16 * LDIM[l], :]
        blk = blk.reshape(NB, 16, LDIM[l], 128).transpose(0, 3, 1, 2)
        nf = np.sum(blk.astype(np.float64) ** 2)
        out[:, :, 0, :, koff[l]:koff[l] + LDIM[l]] = blk / np.sqrt(nf / 16.0)
    return out



# revision 6
# speedup vs baseline: 4.7912x; 4.7912x over previous
"""Trainium2 Bass kernel for nn_CGLayer (gnn_message_passing).

Contract: kernel(**inputs) takes FULL inputs (as reference.setup_inputs()),
returns FULL output [8,128,1,16,9] f32. Internally: data-parallel over the
batch dim across 8 NeuronCores; per core one batch element.

Algebraic reduction (exact):
  X   = conn @ vertices                  (message passing, per batch)
  Y   = mix_nl(cg(X, X))                 (per-node quadratic in X)
  S   = sum_j sph[:, j, :]               (neighbor sum commutes through the
  Z   = mix_rel(cg(Y, S))                 relative-CG stage: x-side is
  out = Z / sqrt(sum Z^2 / 16)            j-independent)

Device pipeline per core — everything node(i)-on-partition:
  A:  X[i,144]     = matmul(lhsT=connT, rhs=vcat), fp32
  S:  Ssum[i,9]    = reduce_j(sph)                       (gpsimd)
  B:  P[i,9984]    = 13 stride-0 DVE pair products, bf16 out, packed
                     symmetry-folded slot layout (W2 host-folds CG x w_nl)
      PT chunks    = DMA xbar transpose (no PE involvement)
      Y^T[i,144]   = 78 bf16 matmuls lhsT=PT-chunk rhs=W2-chunk, PSUM-accum
  C:  P2[i,1296]   = Y^T * Ssum broadcast (1 DVE op), bf16, padded to 1408
      P2T chunks   = DMA xbar transpose
      Z^T[i,144]   = 11 bf16 matmuls lhsT=P2T-chunk rhs=W3-chunk
Host epilogue: unpack e=(l,c,k) columns, global per-l normalization.
"""
import numpy as np
import ml_dtypes
from math import factorial, sqrt

MAXL = 2
CH = 16
NN = 128
NB = 8
LDIM = [1, 3, 5]
FOFF = [0, 16, 64]
NF = 144
SOFF = [0, 1, 4]

# ------------------------------------------------------------- CG tables
def _cg_coeff(j1, m1, j2, m2, j3, m3):
    if m3 != m1 + m2:
        return 0.0
    pre = sqrt((2 * j3 + 1) * factorial(j3 + j1 - j2) * factorial(j3 - j1 + j2)
               * factorial(j1 + j2 - j3) / factorial(j1 + j2 + j3 + 1))
    pre *= sqrt(factorial(j3 + m3) * factorial(j3 - m3) * factorial(j1 - m1)
                * factorial(j1 + m1) * factorial(j2 - m2) * factorial(j2 + m2))
    s = 0.0
    vmin = max(0, j2 - j3 - m1, j1 - j3 + m2)
    vmax = min(j1 + j2 - j3, j1 - m1, j2 + m2)
    for v in range(vmin, vmax + 1):
        s += (-1) ** v / (factorial(v) * factorial(j1 + j2 - j3 - v)
                          * factorial(j1 - m1 - v) * factorial(j2 + m2 - v)
                          * factorial(j3 - j2 + m1 + v) * factorial(j3 - j1 - m2 + v))
    return pre * s


def _cg_matrix(l1, l2, l):
    M = np.zeros((2 * l1 + 1, 2 * l2 + 1, 2 * l + 1))
    for m1 in range(-l1, l1 + 1):
        for m2 in range(-l2, l2 + 1):
            if -l <= m1 + m2 <= l:
                M[m1 + l1, m2 + l2, m1 + m2 + l] = _cg_coeff(l1, m1, l2, m2, l, m1 + m2)
    return M


def _valid_pairs(l):
    return [(l1, l2) for l1 in range(3) for l2 in range(3)
            if abs(l1 - l2) <= l <= l1 + l2]

# ----------------------------------------------------- packed slot layout
# q = (l1, l2, m1) with l1 <= l2; for diagonal pairs m2 >= m1 (symmetric
# fold: the (m2, m1) ordering's weight folds onto the kept slot with the
# channel grid transposed). Slots of one q are contiguous over its valid,
# contiguous m2-range; each (q, m2) block is a 256-slot (c, d) grid.
def _build_qfold():
    q = []
    off = 0
    for l1 in range(3):
        for l2 in range(l1, 3):
            for m1 in range(2 * l1 + 1):
                mt1 = m1 - l1
                lo = max(0, -2 - mt1 + l2)
                hi = min(2 * l2, 2 - mt1 + l2)
                if l1 == l2:
                    lo = max(lo, m1)
                if lo > hi:
                    continue
                n = hi - lo + 1
                q.append(dict(l1=l1, l2=l2, m1=m1, m2_lo=lo, n_m2=n, off=off))
                off += 256 * n
    return q, off

Q_FOLD, NSLOT = _build_qfold()          # 13 ops, 9984 slots
NCHUNK = NSLOT // 128                   # 78
_QIDX = {(e["l1"], e["l2"], e["m1"]): e for e in Q_FOLD}
# pipeline groups: (q-index range) -> chunk counts 24/24/14/16
GROUPS = [(0, 4), (4, 8), (8, 10), (10, 13)]
GSLOTS = [sum(256 * Q_FOLD[i]["n_m2"] for i in range(a, b)) for a, b in GROUPS]
GCHUNK = [s // 128 for s in GSLOTS]
assert sum(GSLOTS) == NSLOT and all(s % 128 == 0 for s in GSLOTS)

NP2 = 9 * NF                            # 1296
NP2PAD = 1408                           # 11 chunks of 128
NCH3 = NP2PAD // 128

_CAR, _DAR = np.meshgrid(np.arange(16), np.arange(16), indexing="ij")


def _assemble_W2(w_nl):
    """W2[NSLOT, 144] f64: folded CG x w_nl; cols e = FOFF[l]+c'*LDIM[l]+k."""
    W2 = np.zeros((NSLOT, NF))
    for l in range(3):
        off_t = 0
        for (p1, p2) in _valid_pairs(l):
            Cg = _cg_matrix(p1, p2, l)
            wl = np.asarray(w_nl[l], np.float64)
            for m1 in range(2 * p1 + 1):
                for m2 in range(2 * p2 + 1):
                    st = (m1 - p1) + (m2 - p2)
                    if abs(st) > l:
                        continue
                    gc = Cg[m1, m2, st + l]
                    if gc == 0.0:
                        continue
                    if (p1 < p2) or (p1 == p2 and m1 <= m2):
                        e_ = _QIDX[(p1, p2, m1)]
                        base = e_["off"] + (m2 - e_["m2_lo"]) * 256
                        slots = base + _CAR * 16 + _DAR
                    else:
                        e_ = _QIDX[(p2, p1, m2)]
                        base = e_["off"] + (m1 - e_["m2_lo"]) * 256
                        slots = base + _DAR * 16 + _CAR
                    t = off_t + _CAR * 16 + _DAR
                    cols = FOFF[l] + np.arange(16) * LDIM[l] + (st + l)
                    W2[np.ix_(slots.ravel(), cols)] += gc * wl[t.ravel(), :]
            off_t += 256
    return W2


def _assemble_W3(w_rel):
    """W3[NP2PAD, 144]: contraction P2[i,(n,e)] -> Z[i,e']; rows n*144+e."""
    W3 = np.zeros((NP2PAD, NF))
    ar = np.arange(16)
    for l in range(3):
        off_t = 0
        for (p1, p2) in _valid_pairs(l):          # p1 = Y side, p2 = sph side
            Cg = _cg_matrix(p1, p2, l)
            wr = np.asarray(w_rel[l], np.float64)
            for m1 in range(2 * p1 + 1):
                for m2 in range(2 * p2 + 1):
                    st = (m1 - p1) + (m2 - p2)
                    if abs(st) > l:
                        continue
                    gc = Cg[m1, m2, st + l]
                    if gc == 0.0:
                        continue
                    n = SOFF[p2] + m2
                    rows = n * NF + FOFF[p1] + ar * LDIM[p1] + m1
                    cols = FOFF[l] + ar * LDIM[l] + (st + l)
                    W3[np.ix_(rows, cols)] += gc * wr[off_t:off_t + 16, :]
            off_t += 16
    return W3

# ------------------------------------------------------------ bass builder
_NC_CACHE = {}


def _build_nc(debug=False):
    import concourse.bacc as bacc
    import concourse.bass as bass
    import concourse.tile as tile
    from concourse import mybir

    f32 = mybir.dt.float32
    bf16 = mybir.dt.bfloat16
    nc = bacc.Bacc()
    d_connT = nc.declare_dram_parameter("connT", [128, 128], f32, isOutput=False)
    d_vcat = nc.declare_dram_parameter("vcat", [128, NF], f32, isOutput=False)
    d_sph = nc.declare_dram_parameter("sph", [128, 128 * 9], f32, isOutput=False)
    d_w2 = nc.declare_dram_parameter("w2", [128, NCHUNK * NF], bf16, isOutput=False)
    d_w3 = nc.declare_dram_parameter("w3", [128, NCH3 * NF], bf16, isOutput=False)
    d_zout = nc.declare_dram_parameter("zout", [128, NF], f32, isOutput=True)
    if debug:
        d_dbgx = nc.declare_dram_parameter("dbgx", [128, NF], f32, isOutput=True)
        d_dbgs = nc.declare_dram_parameter("dbgs", [128, 9], f32, isOutput=True)
        d_dbgp = nc.declare_dram_parameter("dbgp", [128, NSLOT], bf16, isOutput=True)
        d_dbgy = nc.declare_dram_parameter("dbgy", [128, NF], f32, isOutput=True)
        d_dbgp2 = nc.declare_dram_parameter("dbgp2", [128, NP2PAD], bf16, isOutput=True)

    def vap(t, doff, freedims):
        base = t[:] if not isinstance(t, bass.AP) else t
        return bass.AP(tensor=base.tensor, offset=base.offset + doff,
                       ap=[list(base.ap[0])] + [list(d) for d in freedims])

    with tile.TileContext(nc) as tc:
      with (
        tc.tile_pool(name="sb", bufs=1) as sb,
        tc.tile_pool(name="pp", bufs=2) as pp,
        tc.tile_pool(name="ptp", bufs=2) as ptp,
        tc.tile_pool(name="ps_x", bufs=1, space="PSUM") as ps_x,
        tc.tile_pool(name="ps_y", bufs=1, space="PSUM") as ps_y,
        tc.tile_pool(name="ps_z", bufs=1, space="PSUM") as ps_z,
      ):
        # ---- input DMAs
        connT = sb.tile([128, 128], f32)
        nc.sync.dma_start(out=connT, in_=d_connT[:, :])
        vcat = sb.tile([128, NF], f32)
        nc.sync.dma_start(out=vcat, in_=d_vcat[:, :])
        w2 = sb.tile([128, NCHUNK, NF], bf16)
        coff = [0] + list(np.cumsum(GCHUNK))
        for g in range(4):
            s, e = int(coff[g]) * NF, int(coff[g + 1]) * NF
            nc.sync.dma_start(out=vap(w2, s, [[1, e - s]]), in_=d_w2[:, s:e])
        sph = sb.tile([128, 128 * 9], f32)
        for h in range(2):
            nc.sync.dma_start(out=sph[:, h * 576:(h + 1) * 576],
                              in_=d_sph[:, h * 576:(h + 1) * 576])
        w3 = sb.tile([128, NCH3, NF], bf16)
        nc.sync.dma_start(
            out=w3, in_=d_w3[:, :].rearrange("p (c e) -> p c e", c=NCH3, e=NF))

        # ---- stage A: X[i, feat] = connT.T @ vcat
        x_ps = ps_x.tile([128, NF], f32)
        nc.tensor.matmul(x_ps, connT, vcat, start=True, stop=True)
        X = sb.tile([128, NF], f32)
        nc.scalar.activation(X, x_ps, mybir.ActivationFunctionType.Copy)

        # ---- stage S: Ssum[i, 9] = sum_j sph   (gpsimd; off critical path)
        ssum = sb.tile([128, 9], f32)
        nc.vector.tensor_reduce(
            ssum, vap(sph, 0, [[1, 9], [9, 128]]),
            mybir.AxisListType.X, mybir.AluOpType.add)

        # ---- stage B: products (DVE) -> DMA transpose -> bf16 matmuls
        ymix = ps_y.tile([128, NF], f32)
        gchunk = 0
        for g, (qa, qb) in enumerate(GROUPS):
            gslots = GSLOTS[g]
            gbase = Q_FOLD[qa]["off"]
            P = pp.tile([128, GSLOTS[0]], bf16)
            for qi in range(qa, qb):
                op = Q_FOLD[qi]
                l1, l2, m1 = op["l1"], op["l2"], op["m1"]
                nm2 = op["n_m2"]
                nc.vector.tensor_tensor(
                    out=vap(P, op["off"] - gbase, [[256, nm2], [16, 16], [1, 16]]),
                    in0=vap(X, FOFF[l1] + m1, [[0, nm2], [LDIM[l1], 16], [0, 16]]),
                    in1=vap(X, FOFF[l2] + op["m2_lo"],
                            [[1, nm2], [0, 16], [LDIM[l2], 16]]),
                    op=mybir.AluOpType.mult)
            PT = ptp.tile([128, GCHUNK[0], 128], bf16)
            nch = GCHUNK[g]
            nc.sync.dma_start(out=PT[:, 0:nch, :], in_=P[:, 0:gslots],
                              transpose=True)
            if debug:
                nc.sync.dma_start(out=d_dbgp[:, gbase:gbase + gslots],
                                  in_=P[:, 0:gslots])
            for c in range(nch):
                k = gchunk + c
                nc.tensor.matmul(ymix, PT[:, c, :], w2[:, k, :],
                                 start=(k == 0), stop=(k == NCHUNK - 1))
            gchunk += nch

        # ---- stage C: P2 = Y^T * Ssum -> transpose -> 11 matmuls
        P2 = sb.tile([128, NP2PAD], bf16)
        nc.gpsimd.memset(P2[:, NP2:NP2PAD], 0.0)
        nc.vector.tensor_tensor(
            out=vap(P2, 0, [[NF, 9], [1, NF]]),
            in0=vap(ymix, 0, [[0, 9], [1, NF]]),
            in1=vap(ssum, 0, [[1, 9], [0, NF]]),
            op=mybir.AluOpType.mult)
        P2T = sb.tile([128, NCH3, 128], bf16)
        nc.sync.dma_start(out=P2T, in_=P2[:, :], transpose=True)
        z_ps = ps_z.tile([128, NF], f32)
        for c in range(NCH3):
            nc.tensor.matmul(z_ps, P2T[:, c, :], w3[:, c, :],
                             start=(c == 0), stop=(c == NCH3 - 1))
        zsb = sb.tile([128, NF], f32)
        nc.scalar.activation(zsb, z_ps, mybir.ActivationFunctionType.Copy)
        nc.sync.dma_start(out=d_zout[:, :], in_=zsb)

        if debug:
            nc.sync.dma_start(out=d_dbgx[:, :], in_=X)
            nc.sync.dma_start(out=d_dbgs[:, :], in_=ssum)
            ydbg = sb.tile([128, NF], f32)
            nc.vector.tensor_copy(out=ydbg, in_=ymix)
            nc.sync.dma_start(out=d_dbgy[:, :], in_=ydbg)
            nc.sync.dma_start(out=d_dbgp2[:, :], in_=P2)

    nc.compile()
    return nc

# ------------------------------------------------------------- host entry
LAST_RESULT = {}


def _get_nc():
    if "nc" not in _NC_CACHE:
        _NC_CACHE["nc"] = _build_nc()
    return _NC_CACHE["nc"]


def _pack_chunked(W, nchunk):
    """[nchunk*128, e] -> [128, nchunk*e] bf16 (chunk-major per partition)."""
    e = W.shape[1]
    return np.ascontiguousarray(
        W.reshape(nchunk, 128, e).transpose(1, 0, 2)
        .astype(ml_dtypes.bfloat16).reshape(128, nchunk * e))


def kernel(vertices_0, vertices_1, vertices_2, connectivity,
           sph_0, sph_1, sph_2,
           w_nl_0, w_nl_1, w_nl_2,
           w_rel_0, w_rel_1, w_rel_2):
    from concourse.bass_utils import run_bass_kernel_spmd

    f = np.float32
    verts = [np.asarray(v, f) for v in (vertices_0, vertices_1, vertices_2)]
    sphs = [np.asarray(s, f) for s in (sph_0, sph_1, sph_2)]
    conn = np.asarray(connectivity)
    W2 = _assemble_W2([np.asarray(w, f) for w in (w_nl_0, w_nl_1, w_nl_2)])
    W3 = _assemble_W3([np.asarray(w, f) for w in (w_rel_0, w_rel_1, w_rel_2)])
    w2p = _pack_chunked(W2, NCHUNK)
    w3p = _pack_chunked(W3, NCH3)

    in_maps = []
    for b in range(NB):
        connT = np.ascontiguousarray(conn[b].astype(f).T)
        vcat = np.concatenate([v[b].reshape(128, -1) for v in verts], axis=1)
        sph_cat = np.concatenate([s[b][:, :, 0, :] for s in sphs], axis=-1)
        in_maps.append(dict(connT=connT, vcat=np.ascontiguousarray(vcat),
                            sph=np.ascontiguousarray(sph_cat.reshape(128, 128 * 9)),
                            w2=w2p, w3=w3p))

    res = run_bass_kernel_spmd(_get_nc(), in_maps, list(range(NB)))
    LAST_RESULT["res"] = res
    Z = np.stack([res.results[b]["zout"] for b in range(NB)])   # [8, 128, 144]

    # host epilogue: unpack e=(l,c,k) cols, global per-l normalization
    out = np.zeros((NB, 128, 1, 16, 9), dtype=f)
    koff = [0, 1, 4]
    for l in range(3):
        cols = FOFF[l] + (np.arange(16)[:, None] * LDIM[l]
                          + np.arange(LDIM[l])[None, :])
        blk = Z[:, :, cols]                                     # [8,128,16,ld]
        nf = np.sum(blk.astype(np.float64) ** 2)
        out[:, :, 0, :, koff[l]:koff[l] + LDIM[l]] = blk / np.sqrt(nf / 16.0)
    return out
